# revision 8
# baseline (speedup 1.0000x reference)
"""Trainium2 Bass kernel for nn_Detector (region-sum pooling + softmax).

The reference computes softmax(x.reshape(B, H*W) @ filt) where filt is a
fixed 0/1 mask selecting 10 disjoint 113x113 rectangular regions of the
1024x1024 image.  The dense GEMM is really a sparse pooling: out[b, k]
is the sum of x[b] over region k.  Only ~12% of x is ever needed, so we
DMA exactly the 10 regions per image instead of streaming all 512 MB.

Distribution: data-parallel over batch, 8 NeuronCores x 16 images each.

DMA strategy (measured on HW, all 8 cores active): a region row is 452 B
of f32, and every DMA ring is descriptor-rate-limited on this pattern,
not byte-limited.  The three dynamic rings drain CONCURRENTLY: the
SWDGE ring (gpsimd) sustains ~32 ns/descriptor, the two HWDGE rings
(sync = qSPDynamicHW, scalar = qActDynamicHW) ~9 ns/descriptor
combined-per-two-rings -- but HWDGE is only fast for 128-partition DMAs
with >= 512 B descriptors (64-partition or 452 B HWDGE DMAs collapse to
~10x slower gen and poison the engines).  So: 7 regions ride gpsimd
with exact 113-col (452 B) windows, and 3 region-equivalents ride
sync/scalar with 128-col (512 B) windows starting at the 64B-aligned
col s0 = 112*cb (region at tile cols [cb, cb+113)).  Region 8 is split
into rows 0-55 (sync) / 56-111 (scalar) as two 128-partition DMAs with
partition = (img, 7-row group); p // 8 is still the image, so the
standard octet indicator contracts them and the two halves land in
mpart columns 8 and 10, folded together after the matmul.

Per gpsimd region, one DMA loads rows r0..r0+111 for all 16 images:
DRAM side is the monotonic 3D slice x[:, r0:r0+112, c0:c0+113]; SBUF
side is [128, 14, 113] with partition = (batch, row-octet).  Both sides
enumerate elements in the same order, so no AP rearrange is needed
(SWDGE crashes on non-monotonic or 4D APs).  The 113th row of every
region goes via a small HWDGE DMA on the sync queue in parallel.

Compute: VectorE reduces per region -> [128, 1] partials (sliced to the
real 113 cols for the padded tiles); one TensorE matmul with the 0/1
block indicator [128, 16] contracts the 8 octets per batch -> PSUM
[16, 11]; VectorE adds the remainder-row partials and region-8's second
half; ScalarE does the numerically-stable softmax.
"""

import numpy as np

import concourse.bass as bass
import concourse.tile as tile
from concourse import bacc, mybir
from concourse.bass_utils import run_bass_kernel_spmd

# Problem geometry — fixed by the reference's _build_filter(1024, 1024).
B, H, W = 128, 1024, 1024
S = 113  # min(1024 // 9, 1024 // 7)
REGIONS = [(2, 1), (2, 4), (2, 7), (4, 1), (4, 3), (4, 5), (4, 7), (6, 1), (6, 4), (6, 7)]
K = len(REGIONS)
N_CORES = 8
BPC = B // N_CORES  # images per core
F32 = mybir.dt.float32
OCT, GR = 8, 14   # 112 of the 113 region rows = 8 octets x 14 rows
GRC = 128         # padded col window for HWDGE regions (512 B descriptors)
HGR = 7           # row-group height for the split region halves

SW_KS = [0, 1, 2, 3, 4, 5, 6]  # gpsimd/SWDGE ring
K_SYNC, K_SCAL, K_SPLIT = 7, 9, 8


def host_blk():
    # blk[p, b] = 1 iff p // 8 == b: sums the 8 octets per batch.
    return np.repeat(np.eye(BPC, dtype=np.float32), OCT, axis=0)


def build_nc():
    nc = bacc.Bacc("TRN2", target_bir_lowering=False, debug=False)
    x = nc.declare_dram_parameter("x", [BPC, H, W], F32, isOutput=False)
    blk_d = nc.declare_dram_parameter("blk", [128, BPC], F32, isOutput=False)
    out = nc.declare_dram_parameter("out", [BPC, K], F32, isOutput=True)

    with tile.TileContext(nc) as tc:
        with (
            tc.tile_pool(name="reg", bufs=4) as rpool,
            tc.tile_pool(name="small", bufs=1) as spool,
            tc.tile_pool(name="psum", bufs=1, space=bass.MemorySpace.PSUM) as ppool,
        ):
            # HWDGE region DMAs first: their descriptor gen is RTL
            # (~0.8 us), so these rings start draining before Q7 finishes
            # generating the first SWDGE region's descriptors.
            hw_tiles = {}
            for k, eng in ((K_SYNC, nc.sync), (K_SCAL, nc.scalar)):
                rb, cb = REGIONS[k]
                r0, s0 = rb * S, 112 * cb
                mt = spool.tile([128, GR, GRC], F32, tag=f"hw{k}")
                eng.dma_start(out=mt[:], in_=x[:, r0:r0 + OCT * GR, s0:s0 + GRC])
                hw_tiles[k] = mt
            # Region 8 split into two 128-partition halves by 7-row groups.
            rb8, cb8 = REGIONS[K_SPLIT]
            r8, s8 = rb8 * S, 112 * cb8
            mt8a = spool.tile([128, HGR, GRC], F32, tag="hw8a")
            nc.sync.dma_start(
                out=mt8a[:], in_=x[:, r8:r8 + 8 * HGR, s8:s8 + GRC]
            )
            mt8b = spool.tile([128, HGR, GRC], F32, tag="hw8b")
            nc.scalar.dma_start(
                out=mt8b[:], in_=x[:, r8 + 8 * HGR:r8 + 16 * HGR, s8:s8 + GRC]
            )

            # SWDGE ring: the remaining 7 regions, exact 452 B windows.
            sw_tiles = {}
            for k in SW_KS:
                rb, cb = REGIONS[k]
                r0, c0 = rb * S, cb * S
                mt = rpool.tile([128, GR, S], F32, tag="mt")
                nc.gpsimd.dma_start(
                    out=mt[:], in_=x[:, r0:r0 + OCT * GR, c0:c0 + S]
                )
                sw_tiles[k] = mt

            # Remainder row (the 113th) of every region: tiny sync DMAs.
            rem = spool.tile([BPC, K, S], F32)
            for k, (rb, cb) in enumerate(REGIONS):
                r0, c0 = rb * S, cb * S
                nc.sync.dma_start(
                    out=rem[:, k, :], in_=x[:, r0 + OCT * GR, c0:c0 + S]
                )

            # Block indicator (host-provided — engine memsets can only
            # start at partition 0/32/64/96).  After the bulk DMAs so it
            # doesn't delay the sync ring's region drain.
            blk = spool.tile([128, BPC], F32)
            nc.sync.dma_start(out=blk[:], in_=blk_d[:])

            # Per-region partials; issue in estimated DMA arrival order.
            # Column 10 holds region 8's second half.
            mpart = spool.tile([128, K + 1], F32)

            def red(col, tile_, sl=None):
                in_ = tile_[:] if sl is None else tile_[:, :, sl[0]:sl[1]]
                nc.vector.reduce_sum(
                    out=mpart[:, col:col + 1], in_=in_, axis=mybir.AxisListType.XY
                )

            cb7, cb9 = REGIONS[K_SYNC][1], REGIONS[K_SCAL][1]
            red(0, sw_tiles[0])
            red(1, sw_tiles[1])
            red(2, sw_tiles[2])
            red(K_SYNC, hw_tiles[K_SYNC], (cb7, cb7 + S))
            red(K_SCAL, hw_tiles[K_SCAL], (cb9, cb9 + S))
            red(3, sw_tiles[3])
            red(4, sw_tiles[4])
            red(K_SPLIT, mt8a, (cb8, cb8 + S))
            red(10, mt8b, (cb8, cb8 + S))
            red(5, sw_tiles[5])
            red(6, sw_tiles[6])

            rpart = spool.tile([BPC, K], F32)
            nc.vector.reduce_sum(out=rpart[:], in_=rem[:], axis=mybir.AxisListType.X)

            # Contract the 8 octets (or 8 row-groups) per batch.
            py = ppool.tile([BPC, K + 1], F32)
            nc.tensor.matmul(py[:], blk[:], mpart[:], start=True, stop=True)

            ys = spool.tile([BPC, K], F32)
            nc.vector.tensor_add(ys[:], py[:, 0:K], rpart[:])
            nc.vector.tensor_add(ys[:, 8:9], ys[:, 8:9], py[:, 10:11])

            # Softmax over the 10 detectors, batches on partitions.
            m = spool.tile([BPC, 1], F32)
            nc.vector.reduce_max(m[:], ys[:], axis=mybir.AxisListType.X)
            negm = spool.tile([BPC, 1], F32)
            nc.vector.tensor_scalar_mul(negm[:], m[:], -1.0)
            e = spool.tile([BPC, K], F32)
            ssum = spool.tile([BPC, 1], F32)
            nc.scalar.activation(
                e[:], ys[:], mybir.ActivationFunctionType.Exp,
                bias=negm[:], accum_out=ssum[:],
            )
            rcp = spool.tile([BPC, 1], F32)
            nc.vector.reciprocal(rcp[:], ssum[:])
            o = spool.tile([BPC, K], F32)
            nc.scalar.mul(o[:], e[:], rcp[:])
            nc.sync.dma_start(out=out[:], in_=o[:])

    nc.compile()
    return nc


_NC = None


def get_nc():
    global _NC
    if _NC is None:
        _NC = build_nc()
    return _NC


def kernel(x, filt=None, **_unused):
    nc = get_nc()
    x = np.ascontiguousarray(np.asarray(x, dtype=np.float32))
    assert x.shape == (B, H, W), x.shape
    blk = host_blk()
    in_maps = [
        {"x": x[i * BPC:(i + 1) * BPC], "blk": blk} for i in range(N_CORES)
    ]
    res = run_bass_kernel_spmd(nc, in_maps, list(range(N_CORES)))
    return np.concatenate([r["out"] for r in res.results], axis=0)


# revision 9
# speedup vs baseline: 1.5524x; 1.5524x over previous
"""Trainium2 Bass kernel for nn_Detector (region-sum pooling + softmax).

The reference computes softmax(x.reshape(B, H*W) @ filt) where filt is a
fixed 0/1 mask selecting 10 disjoint 113x113 rectangular regions of the
1024x1024 image.  The dense GEMM is really a sparse pooling: out[b, k]
is the sum of x[b] over region k.  Only ~12% of x is ever needed, so we
DMA exactly the 10 regions per image instead of streaming all 512 MB.

Distribution: data-parallel over batch, 8 NeuronCores x 16 images each.

All bulk loads ride the SWDGE (gpsimd) ring: a region row is one 452 B
descriptor and the ring is descriptor-rate-limited (~32 ns/desc across
16 SDMA engines) regardless of padding, alignment, or dtype, so exact
113-col windows minimize both descriptors and bytes.  Spreading regions
onto the HWDGE rings (sync/scalar) measures WORSE chip-wide (ring
mixing collapses all rings' packet rates), so everything stays on one
ring.  All 10 descriptor-generation instructions issue back-to-back on
Q7 (bufs=11, no tile-pool reuse stalls); each gen is ~1.3 us, well
ahead of the ~3.6 us/region drain.

Per region, one SWDGE DMA loads rows r0..r0+111 for all 16 images:
DRAM side is the plain monotonic 3D slice x[:, r0:r0+112, c0:c0+113];
SBUF side is [128, 14, 113] with partition = (batch, row-octet).  Both
sides enumerate elements in the same order, so no AP rearrange is
needed (SWDGE crashes on non-monotonic or 4D APs).  The last region is
split into two half-height DMAs (partition = (batch, 7-row group)) so
the final reduce only covers half a region after the last packet lands.
The 113th row of every region goes via small HWDGE DMAs on the sync
queue in parallel.

Compute: VectorE reduces each region tile -> [128, 1] partials in DMA
arrival order; one TensorE matmul with the 0/1 block indicator
[128, 16] contracts the 8 octets (or row-groups) per batch -> PSUM
[16, 11]; VectorE folds the split region's second half and adds the
remainder-row partials; ScalarE does the numerically-stable softmax.
"""

import numpy as np

import concourse.bass as bass
import concourse.tile as tile
from concourse import bacc, mybir
from concourse.bass_utils import run_bass_kernel_spmd

# Problem geometry — fixed by the reference's _build_filter(1024, 1024).
B, H, W = 128, 1024, 1024
S = 113  # min(1024 // 9, 1024 // 7)
REGIONS = [(2, 1), (2, 4), (2, 7), (4, 1), (4, 3), (4, 5), (4, 7), (6, 1), (6, 4), (6, 7)]
K = len(REGIONS)
N_CORES = 8
BPC = B // N_CORES  # images per core
F32 = mybir.dt.float32
OCT, GR = 8, 14  # 112 of the 113 region rows = 8 octets x 14 rows
HGR = 7          # row-group height for the split last region
K_SPLIT = K - 1


def host_blk():
    # blk[p, b] = 1 iff p // 8 == b: sums the 8 octets per batch.
    return np.repeat(np.eye(BPC, dtype=np.float32), OCT, axis=0)


def build_nc():
    nc = bacc.Bacc("TRN2", target_bir_lowering=False, debug=False)
    x = nc.declare_dram_parameter("x", [BPC, H, W], F32, isOutput=False)
    blk_d = nc.declare_dram_parameter("blk", [128, BPC], F32, isOutput=False)
    out = nc.declare_dram_parameter("out", [BPC, K], F32, isOutput=True)

    with tile.TileContext(nc) as tc:
        with (
            tc.tile_pool(name="reg", bufs=11) as rpool,
            tc.tile_pool(name="small", bufs=1) as spool,
            tc.tile_pool(name="psum", bufs=1, space=bass.MemorySpace.PSUM) as ppool,
        ):
            # All SWDGE region DMAs up front: Q7 generates descriptors
            # back-to-back while the SDMA engines drain behind it.
            tiles = []
            for k in range(K - 1):
                rb, cb = REGIONS[k]
                r0, c0 = rb * S, cb * S
                mt = rpool.tile([128, GR, S], F32, tag="mt")
                nc.gpsimd.dma_start(
                    out=mt[:], in_=x[:, r0:r0 + OCT * GR, c0:c0 + S]
                )
                tiles.append(mt)
            # Last region in two half-height DMAs: partition = (img,
            # 7-row group), so p // 8 is still the image and the halves
            # land in mpart columns 9 and 10.
            rb, cb = REGIONS[K_SPLIT]
            r9, c9 = rb * S, cb * S
            mt9a = rpool.tile([128, HGR, S], F32, tag="mt")
            nc.gpsimd.dma_start(
                out=mt9a[:], in_=x[:, r9:r9 + 8 * HGR, c9:c9 + S]
            )
            mt9b = rpool.tile([128, HGR, S], F32, tag="mt")
            nc.gpsimd.dma_start(
                out=mt9b[:], in_=x[:, r9 + 8 * HGR:r9 + 16 * HGR, c9:c9 + S]
            )

            # Remainder row (the 113th) of every region: tiny sync DMAs.
            rem = spool.tile([BPC, K, S], F32)
            for k, (rb, cb) in enumerate(REGIONS):
                r0, c0 = rb * S, cb * S
                nc.sync.dma_start(
                    out=rem[:, k, :], in_=x[:, r0 + OCT * GR, c0:c0 + S]
                )

            # Block indicator (host-provided — engine memsets can only
            # start at partition 0/32/64/96).
            blk = spool.tile([128, BPC], F32)
            nc.sync.dma_start(out=blk[:], in_=blk_d[:])

            # Per-region partials in DMA arrival order; columns 9 and 10
            # hold the split region's halves.
            mpart = spool.tile([128, K + 1], F32)
            for k in range(K - 1):
                nc.vector.reduce_sum(
                    out=mpart[:, k:k + 1], in_=tiles[k][:],
                    axis=mybir.AxisListType.XY,
                )
            rpart = spool.tile([BPC, K], F32)
            nc.vector.reduce_sum(out=rpart[:], in_=rem[:], axis=mybir.AxisListType.X)
            nc.vector.reduce_sum(
                out=mpart[:, 9:10], in_=mt9a[:], axis=mybir.AxisListType.XY
            )
            nc.vector.reduce_sum(
                out=mpart[:, 10:11], in_=mt9b[:], axis=mybir.AxisListType.XY
            )

            # Contract the 8 octets (or row-groups) per batch.
            py = ppool.tile([BPC, K + 1], F32)
            nc.tensor.matmul(py[:], blk[:], mpart[:], start=True, stop=True)

            ys = spool.tile([BPC, K], F32)
            nc.vector.tensor_add(ys[:], py[:, 0:K], rpart[:])
            nc.vector.tensor_add(ys[:, 9:10], ys[:, 9:10], py[:, 10:11])

            # Softmax over the 10 detectors, batches on partitions.
            m = spool.tile([BPC, 1], F32)
            nc.vector.reduce_max(m[:], ys[:], axis=mybir.AxisListType.X)
            negm = spool.tile([BPC, 1], F32)
            nc.vector.tensor_scalar_mul(negm[:], m[:], -1.0)
            e = spool.tile([BPC, K], F32)
            ssum = spool.tile([BPC, 1], F32)
            nc.scalar.activation(
                e[:], ys[:], mybir.ActivationFunctionType.Exp,
                bias=negm[:], accum_out=ssum[:],
            )
            rcp = spool.tile([BPC, 1], F32)
            nc.vector.reciprocal(rcp[:], ssum[:])
            o = spool.tile([BPC, K], F32)
            nc.scalar.mul(o[:], e[:], rcp[:])
            nc.sync.dma_start(out=out[:], in_=o[:])

    nc.compile()
    return nc


_NC = None


def get_nc():
    global _NC
    if _NC is None:
        _NC = build_nc()
    return _NC


def kernel(x, filt=None, **_unused):
    nc = get_nc()
    x = np.ascontiguousarray(np.asarray(x, dtype=np.float32))
    assert x.shape == (B, H, W), x.shape
    blk = host_blk()
    in_maps = [
        {"x": x[i * BPC:(i + 1) * BPC], "blk": blk} for i in range(N_CORES)
    ]
    res = run_bass_kernel_spmd(nc, in_maps, list(range(N_CORES)))
    return np.concatenate([r["out"] for r in res.results], axis=0)
